# revision 1
# baseline (speedup 1.0000x reference)
"""Trainium2 Bass kernel for AxialSelfAttention2d (see reference in module docstring).

Reference computation (per batch b):
    qkv = W @ x + b            (1x1 conv; W [3E, E], x [E, S, L], E = 512)
    q, k, v split; q *= Dh**-0.5; per head h: q,k,v [Dh=64, S, L]
    col:  scores[s,t|l] = q[:,s,l].k[:,t,l]; softmax over t; out_col = attn @ v
    row:  scores[l,m|s] = q[:,s,l].k[:,s,m]; softmax over m; out_row = attn @ v
    out = out_col + out_row    -> [H*Dh, S, L]

Sharding: 8 cores = 2 batches x 4 head-pairs. Each core computes 2 heads of one
batch end-to-end (no collectives); the host concatenates core outputs.

Per-core dataflow (matmul operands fp16, fp32 PSUM accumulation):
  A)  x fp32 --cast-DMA--> SBUF fp16 tiles; QKV projection with W^T stationary
      -> q2, k2, v_sl [128(2h x 64d), S*L] fp16 (+ bias, q pre-scaled on host).
  A2) v_ls = v_sl reordered to (l,s) free order (gpsimd copy);
      vT_row[h][l, s*65+{d,1}] <- DMA-transpose(v_sl[h]);
      vT_col[h][s, l*65+{d,1}] <- DMA-transpose(v_ls[h]); ones columns memset.
  B)  col attention per (l, h): scoresT[t,s] = k_l^T @ q_l (PE, K=64, two heads
      row-packed via base partitions); e = exp(scoresT) (ACT, no max-subtraction
      -- scores are ~N(0,1)); AV: out[s, 65] = e^T.T @ vT_col_l (column 64 gives
      the softmax denominator); fused DVE divide (denominator broadcast with a
      step-0 free dim) -> col_src[s, l*128+hd].
  B2) DMA-transpose col_src chunks -> dst[hd, s*128+l] (final orientation).
  C)  row attention symmetric -> row_src[l, s*128+hd]; DMA-transpose chunks;
      DVE add into dst; cast-DMA (fp16 -> fp32) to DRAM out.
"""

import numpy as np
from contextlib import ExitStack

NUM_HEADS = 8
DIM_HEAD = 64
EMBED = 512
B, S, L = 2, 128, 128
SL = S * L
N_CORES = 8
HPC = 2  # heads per core

_CACHE = {}


def build_program(nc, tc):
    import concourse.bass as bass
    import concourse.mybir as mybir

    f16 = mybir.dt.float16
    f32 = mybir.dt.float32
    AF = mybir.ActivationFunctionType
    OP = mybir.AluOpType
    AP = bass.AP

    x_d = nc.dram_tensor("x", [EMBED, S, L], f32, kind="ExternalInput")
    w_d = nc.dram_tensor("wT", [EMBED, 384], f16, kind="ExternalInput")
    b_d = nc.dram_tensor("bvec", [384], f32, kind="ExternalInput")
    out_d = nc.dram_tensor("out", [128, S, L], f32, kind="ExternalOutput")

    x_flat = x_d.ap().rearrange("c s l -> c (s l)")

    CH = 32          # slice indices per chunk
    NCH = 128 // CH  # 4

    def stage_a(qk_pool, q2, k2, v_sl):
        GW = 2048  # spatial columns per x load
        with tc.tile_pool(name="xload", bufs=2) as xpool, \
             tc.tile_pool(name="wpool", bufs=1) as wpool, \
             tc.tile_pool(name="qkvps", bufs=4, space="PSUM") as qkv_ps:
            w_sb = wpool.tile([128, 4, 384], f16, tag="w")
            nc.sync.dma_start(w_sb[:],
                              w_d.ap().rearrange("(k c) o -> c k o", k=4))
            b_sb = wpool.tile([128, 3], f32, tag="b")
            nc.sync.dma_start(b_sb[:], b_d.ap().rearrange("(m p) -> p m", p=128))
            for g in range(SL // GW):
                xt = xpool.tile([128, 4, GW], f16, tag="x")
                nc.gpsimd.dma_start(
                    xt[:],
                    x_flat[:, g * GW:(g + 1) * GW]
                        .rearrange("(k c) n -> c k n", k=4))
                for m in range(3):  # 0=q, 1=k, 2=v
                    dest = (q2, k2, v_sl)[m]
                    for sg in range(GW // 512):
                        ps = qkv_ps.tile([128, 512], f32, tag="acc")
                        for c in range(4):
                            nc.tensor.matmul(
                                ps[:],
                                w_sb[:][:, c, m * 128:(m + 1) * 128],
                                xt[:][:, c, sg * 512:(sg + 1) * 512],
                                start=(c == 0), stop=(c == 3))
                        off = g * GW + sg * 512
                        nc.vector.tensor_scalar_add(
                            dest[:][:, off:off + 512], ps[:],
                            b_sb[:][:, m:m + 1])

    def make_vt(pool, tmp_pool, tagp, src, n_outer):
        """vt[h][p, i*65 + {0..63: d, 64: 1}] <- transpose of src[h-slice].

        DMA-transpose requires a packed [p, mid, last] output (strided mid
        corrupts data on HW), so transpose into a packed tmp then let gpsimd
        restride into the 65-wide augmented layout."""
        vts = []
        for h in range(HPC):
            vt = pool.tile([128, n_outer * 65], f16, tag=f"{tagp}{h}")
            for qtr in range(n_outer // 32):
                tmp = tmp_pool.tile([128, 32, 64], f16, tag="vtmp")
                nc.sync.dma_start(
                    tmp[:],
                    src[:][h * 64:(h + 1) * 64,
                           qtr * 32 * 128:(qtr + 1) * 32 * 128],
                    transpose=True)
                o = AP(vt[:].tensor, vt[:].offset + qtr * 32 * 65,
                       [list(vt[:].ap[0]), [65, 32], [1, 64]])
                nc.gpsimd.tensor_copy(o, tmp[:])
            ones_ap = AP(vt[:].tensor, vt[:].offset + 64,
                         [list(vt[:].ap[0]), [65, n_outer], [1, 1]])
            nc.vector.memset(ones_ap, 1.0)
            vts.append(vt)
        return vts

    # ---------------- attention (direction 0 = col, 1 = row) ----------------
    def attention(direction, vt, qv, kv, dst, zero_sb):
        with ExitStack() as dctx:
            src_pool = dctx.enter_context(
                tc.tile_pool(name=f"src{direction}", bufs=2))
            sc_ps = dctx.enter_context(
                tc.tile_pool(name=f"scps{direction}", bufs=2, space="PSUM"))
            av_ps = dctx.enter_context(
                tc.tile_pool(name=f"avps{direction}", bufs=2, space="PSUM"))
            e_pool = dctx.enter_context(
                tc.tile_pool(name=f"e{direction}", bufs=4))
            den_pool = dctx.enter_context(
                tc.tile_pool(name=f"den{direction}", bufs=2))
            tr_pool = None
            if direction == 0:
                tr_pool = dctx.enter_context(tc.tile_pool(name="coltr", bufs=2))

            if direction == 0:
                def qk_slice(t, h, i):  # [64, t/s] column i, stride L
                    return t[h * 64:(h + 1) * 64, :, i]
            else:
                def qk_slice(t, h, i):  # [64, m/l] row i, contiguous
                    return t[h * 64:(h + 1) * 64, i, :]

            # PSUM-bank discipline: matmuls with different tile_positions must
            # never write the same bank (HW fault) -> per-head score banks.
            for ch in range(NCH):
                src = src_pool.tile([128, CH * 128], f16, tag="src")
                for quad in range(CH // 4):
                    i0 = ch * CH + quad * 4
                    for h in range(2):
                        sc = sc_ps.tile([128, 512], f32, tag=f"sc{h}")
                        for j in range(4):
                            nc.tensor.matmul(
                                sc[:][:, j * 128:(j + 1) * 128],
                                qk_slice(kv, h, i0 + j),
                                qk_slice(qv, h, i0 + j),
                                start=True, stop=True)
                        et = e_pool.tile([128, 512], f16, tag="e")
                        nc.scalar.activation(et[:], sc[:], AF.Exp,
                                             bias=zero_sb[:][:, 0:1])
                        av = av_ps.tile([128, 260], f32, tag="av")
                        for j in range(4):
                            nc.tensor.matmul(
                                av[:][:, j * 65:(j + 1) * 65],
                                et[:][:, j * 128:(j + 1) * 128],
                                vt[h][:][:, (i0 + j) * 65:(i0 + j + 1) * 65],
                                start=True, stop=True)
                        den = den_pool.tile([128, 4], f32, tag="den")
                        nc.vector.reciprocal(
                            den[:], AP(av[:].tensor, av[:].offset + 64,
                                       [list(av[:].ap[0]), [65, 4]]))
                        # src[s, (i0+j)*128 + h*64 + d] = av[:, j*65+d]*rden[:, j]
                        in0 = AP(av[:].tensor, av[:].offset,
                                 [list(av[:].ap[0]), [65, 4], [1, 64]])
                        in1 = AP(den[:].tensor, den[:].offset,
                                 [list(den[:].ap[0]), [1, 4], [0, 64]])
                        o = AP(src[:].tensor,
                               src[:].offset + (quad * 4) * 128 + h * 64,
                               [list(src[:].ap[0]), [128, 4], [1, 64]])
                        nc.vector.tensor_tensor(o, in0, in1, OP.mult)

                if direction == 1:
                    # row runs first: transpose lands directly in dst
                    # dst[hd, (ch*CH+sr)*128 + l'] <- src[l', sr*128+hd]
                    od = AP(dst[:].tensor, dst[:].offset + ch * CH * 128,
                            [list(dst[:].ap[0]), [128, CH], [1, 128]])
                    nc.sync.dma_start(od, src[:], transpose=True)
                else:
                    # col: transpose to trc[hd, lr*128 + s], then strided add
                    tr = tr_pool.tile([128, CH * 128], f16, tag="tr")
                    ot = AP(tr[:].tensor, tr[:].offset,
                            [list(tr[:].ap[0]), [128, CH], [1, 128]])
                    nc.sync.dma_start(ot, src[:], transpose=True)
                    # dst[hd, s*128 + (ch*CH+lr)] += trc[hd, lr*128 + s]
                    dseg = AP(dst[:].tensor, dst[:].offset + ch * CH,
                              [list(dst[:].ap[0]), [1, CH], [128, S]])
                    nc.vector.tensor_add(dseg, dseg, tr[:])

    # ---------------- top-level pool nesting (LIFO) ----------------
    import os
    stage = os.environ.get("AXIAL_DEBUG_STAGE", "full")
    with tc.tile_pool(name="qk", bufs=1) as qk_pool, \
         tc.tile_pool(name="vt", bufs=1) as vt_pool:
        q2 = qk_pool.tile([128, SL], f16, tag="q2")
        k2 = qk_pool.tile([128, SL], f16, tag="k2")
        zero_sb = qk_pool.tile([128, 1], f32, tag="z")
        nc.vector.memset(zero_sb[:], 0.0)

        with tc.tile_pool(name="vsl", bufs=1) as vsl_pool:
            v_sl = vsl_pool.tile([128, SL], f16, tag="v_sl")
            stage_a(qk_pool, q2, k2, v_sl)
            vt_row = vt_col = None
            if stage != "a":
                with tc.tile_pool(name="vtmp", bufs=2) as tmp_pool:
                    vt_row = make_vt(vt_pool, tmp_pool, "vtr", v_sl, S)
                    with tc.tile_pool(name="vls", bufs=1) as vls_pool:
                        v_ls = vls_pool.tile([128, SL], f16, tag="v_ls")
                        nc.gpsimd.tensor_copy(
                            v_ls[:].rearrange("p (l s) -> p l s", s=S),
                            v_sl[:].rearrange("p (s l) -> p l s", l=L))
                        vt_col = make_vt(vt_pool, tmp_pool, "vtc", v_ls, L)

        with tc.tile_pool(name="dstp", bufs=1) as dst_pool:
            dst = dst_pool.tile([128, SL], f16, tag="dst")  # [hd, s*128+l]
            qv = q2[:].rearrange("p (s l) -> p s l", l=L)
            kv = k2[:].rearrange("p (s l) -> p s l", l=L)
            if stage in ("row", "full"):
                attention(1, vt_row, qv, kv, dst, zero_sb)  # row: fills dst
            if stage == "full":
                attention(0, vt_col, qv, kv, dst, zero_sb)  # col: adds
            if stage in ("a", "a2"):
                nc.vector.tensor_copy(dst[:], q2[:])
            for ch in range(NCH):
                nc.gpsimd.dma_start(
                    out_d.ap()[:, ch * CH:(ch + 1) * CH, :],
                    dst[:][:, ch * CH * 128:(ch + 1) * CH * 128]
                        .rearrange("p (s l) -> p s l", l=L))


def _get_nc():
    if "nc" in _CACHE:
        return _CACHE["nc"]
    import concourse.bacc as bacc
    import concourse.tile as tile

    nc = bacc.Bacc(None, target_bir_lowering=False, debug=False,
                   num_devices=N_CORES)
    with tile.TileContext(nc) as tc:
        build_program(nc, tc)
    nc.compile()
    _CACHE["nc"] = nc
    return nc


def make_in_maps(x, W, b):
    x = np.asarray(x, dtype=np.float32)
    W = np.asarray(W, dtype=np.float32)
    b = np.asarray(b, dtype=np.float32)
    scale = np.float32(DIM_HEAD ** -0.5)
    in_maps = []
    for c in range(N_CORES):
        bb, h0 = c // 4, 2 * (c % 4)
        hd = np.arange(h0 * 64, (h0 + 2) * 64)
        sel = np.concatenate([hd, EMBED + hd, 2 * EMBED + hd])
        W_loc = W[sel, :].copy()
        b_loc = b[sel].copy()
        W_loc[:128] *= scale
        b_loc[:128] *= scale
        in_maps.append({
            "x": np.ascontiguousarray(x[bb]),
            "wT": np.ascontiguousarray(W_loc.T).astype(np.float16),
            "bvec": b_loc.astype(np.float32),
        })
    return in_maps


def assemble(results):
    out = np.empty((B, EMBED, S, L), dtype=np.float32)
    for c, r in enumerate(results):
        bb, h0 = c // 4, 2 * (c % 4)
        out[bb, h0 * 64:(h0 + 2) * 64] = r["out"]
    return out


def kernel(x, W, b):
    from concourse.bass_utils import run_bass_kernel_spmd
    nc = _get_nc()
    res = run_bass_kernel_spmd(nc, make_in_maps(x, W, b),
                               core_ids=list(range(N_CORES)))
    return assemble(res.results)



# revision 41
# speedup vs baseline: 6843.9920x; 6843.9920x over previous
"""Trainium2 Bass kernel for AxialSelfAttention2d.

Reference computation (per batch b):
    qkv = W @ x + b            (1x1 conv; W [3E, E], x [E, S, L], E = 512)
    q, k, v split; q *= Dh**-0.5; per head h: q,k,v [Dh=64, S, L]
    col:  scores[s,t|l] = q[:,s,l].k[:,t,l]; softmax over t; out_col = attn @ v
    row:  scores[l,m|s] = q[:,s,l].k[:,s,m]; softmax over m; out_row = attn @ v
    out = out_col + out_row    -> [H*Dh, S, L]

Sharding: 8 cores = 2 batches x 4 head-pairs. Each core computes 2 heads of
one batch end-to-end (no collectives); the host concatenates core outputs.

Per-core dataflow (matmul operands fp16, fp32 PSUM accumulation):
  A)  x fp32 --cast-DMA--> SBUF fp16 (half-group tiles, 3-deep prefetch);
      QKV projection with W^T stationary -> q2, k2 [128(2h x 64d), S*L].
      v lands in a 2-slab ring (16 s-rows per slab); as each slab completes
      it is DMA-transposed into vt_row[h][l, s16, sr, d] and gpsimd
      (l,s)-reordered into v_ls [hd, l*128+s], so stage A ends with all v
      layouts ready. PSUM evacuation (+bias; q pre-scaled on host) is split
      across DVE (tensor_scalar_add) and ACT (activation Identity with
      per-partition bias) to balance engines.
  B)  row attention per (s, h): scT[m,l'] = k_s^T @ q_s (PE, K=64, heads
      concurrent via base-partition row split); e = exp(scT) (ACT, no
      max-subtraction -- scores ~N(0,1)); AV: av[:, h, j*65+d] = e^T.T @
      vt_row slice plus an N=1 matmul against a ones column (same stationary
      weights) giving the softmax denominator at col j*65+64. Both heads
      share one 2-bank av tile (AV lhsT=et spans all partitions -> same
      tile_position), so one DVE reciprocal + one fused 4-dim-AP divide
      covers a whole quad -> src[l', X*128+hd]; DMA-transpose ->
      dst[hd, s*128+l] (final orientation).
  C)  col attention symmetric (vt_col built from v_ls by 16 packed
      DMA-transposes overlapping the row phase) -> src[s', X*128+hd];
      DMA-transpose -> tr[hd, lr*128+s']; strided adds merge into dst,
      split per 32-s block across DVE/gpsimd (last chunk all-DVE so the
      fp16 output DMA pipelines right behind it). Host casts fp16 -> fp32.
"""

import numpy as np

NUM_HEADS = 8
DIM_HEAD = 64
EMBED = 512
B, S, L = 2, 128, 128
SL = S * L
N_CORES = 8
HPC = 2  # heads per core

# Which engine evacuates each QKV projection output from PSUM.
# "dve" | "act" | "alt" (alternate by op index)
EVAC_PLAN = {"q": "dve", "k": "act", "v": "act"}

_CACHE = {}


def build_program(nc, tc):
    import concourse.bass as bass
    import concourse.mybir as mybir

    f16 = mybir.dt.float16
    f32 = mybir.dt.float32
    AF = mybir.ActivationFunctionType
    OP = mybir.AluOpType
    AP = bass.AP

    x_d = nc.dram_tensor("x", [EMBED, S, L], f32, kind="ExternalInput")
    w_d = nc.dram_tensor("wT", [EMBED, 384], f16, kind="ExternalInput")
    b_d = nc.dram_tensor("bvec", [384], f32, kind="ExternalInput")
    out_d = nc.dram_tensor("out", [128, S, L], f16, kind="ExternalOutput")

    x_flat = x_d.ap().rearrange("c s l -> c (s l)")
    out_flat = out_d.ap().rearrange("c s l -> c (s l)")

    GW = 2048         # spatial columns per x load (16 s-values)
    NG = SL // GW     # 8
    CH = 16           # i-values per attention output chunk
    NCH = 128 // CH   # 8

    import os
    stage = os.environ.get("AXIAL_DEBUG_STAGE", "full")

    def evac(kind, idx, dest_ap, ps_ap, bias_ap):
        plan = EVAC_PLAN[kind]
        use_act = plan == "act" or (plan == "alt" and idx % 2 == 0)
        if use_act:
            nc.scalar.activation(dest_ap, ps_ap, AF.Identity, bias=bias_ap)
        else:
            nc.vector.tensor_scalar_add(dest_ap, ps_ap, bias_ap)

    # ---------------- attention chunk (direction 0 = col, 1 = row) ----------
    def attention_chunk(direction, ch, vt, qv, kv, dst, zero_sb, ones_sb,
                        pools):
        src_pool, tr_pool, sc_ps, av_ps, e_pool, den_pool = pools

        if direction == 0:
            def qk_slice(t, h, i):  # [64, t/s] column i, stride L
                return t[h * 64:(h + 1) * 64, :, i]
        else:
            def qk_slice(t, h, i):  # [64, m/l] row i, contiguous
                return t[h * 64:(h + 1) * 64, i, :]

        # PSUM-bank discipline: matmuls with different tile_positions must
        # never write the same bank (HW fault) -> per-head score/av banks.
        # Heads run concurrently on PE row-halves (lhsT base partition).
        src = src_pool.tile([128, CH * 128], f16, tag="src", name="src")
        for quad in range(CH // 4):
            i0 = ch * CH + quad * 4
            for h in range(2):
                sc = sc_ps.tile([128, 512], f32, tag=f"sc{h}", name="sc")
                for j in range(4):
                    nc.tensor.matmul(
                        sc[:][:, j * 128:(j + 1) * 128],
                        qk_slice(kv, h, i0 + j),
                        qk_slice(qv, h, i0 + j),
                        start=True, stop=True)
                et = e_pool.tile([128, 512], f16, tag="et", name="et")
                nc.scalar.activation(et[:], sc[:], AF.Exp,
                                     bias=zero_sb[:][:, 0:1])
                av = av_ps.tile([128, 260], f32, tag=f"av{h}", name="av")
                for j in range(4):
                    i = i0 + j
                    nc.tensor.matmul(
                        av[:][:, j * 65:j * 65 + 64],
                        et[:][:, j * 128:(j + 1) * 128],
                        vt[h][:][:, i // 16, i % 16, :],
                        start=True, stop=True)
                    nc.tensor.matmul(
                        av[:][:, j * 65 + 64:j * 65 + 65],
                        et[:][:, j * 128:(j + 1) * 128],
                        ones_sb[:][:, 0:1],
                        start=True, stop=True)
                den = den_pool.tile([128, 4], f32, tag=f"den{h}", name="den")
                nc.vector.reciprocal(
                    den[:], AP(av[:].tensor, av[:].offset + 64,
                               [list(av[:].ap[0]), [65, 4]]))
                # src[p, (quad*4+j)*128 + h*64 + d] = av[:, j*65+d]*rden[:, j]
                in0 = AP(av[:].tensor, av[:].offset,
                         [list(av[:].ap[0]), [65, 4], [1, 64]])
                in1 = AP(den[:].tensor, den[:].offset,
                         [list(den[:].ap[0]), [1, 4], [0, 64]])
                o = AP(src[:].tensor,
                       src[:].offset + (quad * 4) * 128 + h * 64,
                       [list(src[:].ap[0]), [128, 4], [1, 64]])
                nc.vector.tensor_tensor(o, in0, in1, OP.mult)

        if direction == 1:
            # row: transpose lands directly in dst
            # dst[hd, (ch*CH+X)*128 + l'] <- src[l', X*128+hd]
            od = AP(dst[:].tensor, dst[:].offset + ch * CH * 128,
                    [list(dst[:].ap[0]), [128, CH], [1, 128]])
            nc.sync.dma_start(od, src[:], transpose=True)
        else:
            # col: transpose to tr[hd, lr*128 + s'], then strided adds
            # dst[hd, s'*128 + ch*CH + lr] += tr[hd, lr*128 + s']
            # split by 32-s block; last chunk on DVE so the out DMA can
            # pipeline immediately behind each block's final merge.
            tr = tr_pool.tile([128, CH * 128], f16, tag="tr", name="tr")
            ot = AP(tr[:].tensor, tr[:].offset,
                    [list(tr[:].ap[0]), [128, CH], [1, 128]])
            nc.sync.dma_start(ot, src[:], transpose=True)
            for sb in range(4):
                dseg = AP(dst[:].tensor,
                          dst[:].offset + sb * 32 * 128 + ch * CH,
                          [list(dst[:].ap[0]), [1, CH], [128, 32]])
                tin = AP(tr[:].tensor, tr[:].offset + sb * 32,
                         [list(tr[:].ap[0]), [128, CH], [1, 32]])
                if ch == NCH - 1:
                    if ch == NCH - 1 or sb % 2 == 0:
                    nc.vector.tensor_tensor(dseg, dseg, tin, OP.add)
                else:
                    nc.gpsimd.tensor_tensor(dseg, dseg, tin, OP.add)
                else:
                    nc.gpsimd.tensor_tensor(dseg, dseg, tin, OP.add)

    # ---------------- top-level pools ----------------
    with tc.tile_pool(name="base", bufs=1) as base_pool, \
         tc.tile_pool(name="vtrp", bufs=1) as vtr_pool, \
         tc.tile_pool(name="vtcp", bufs=1) as vtc_pool, \
         tc.tile_pool(name="dstp", bufs=1) as dst_pool, \
         tc.tile_pool(name="srcp", bufs=2) as src_pool, \
         tc.tile_pool(name="ep", bufs=1) as e_pool, \
         tc.tile_pool(name="denp", bufs=4) as den_pool:
        q2 = base_pool.tile([128, SL], f16, tag="q2")
        k2 = base_pool.tile([128, SL], f16, tag="k2")
        zero_sb = base_pool.tile([128, 1], f32, tag="z")
        nc.vector.memset(zero_sb[:], 0.0)
        ones_sb = base_pool.tile([128, 1], f16, tag="one")
        nc.vector.memset(ones_sb[:], 1.0)
        vtr = [vtr_pool.tile([128, 8, 16, 64], f16, tag=f"vtr{h}",
                             name=f"vtr{h}") for h in range(HPC)]
        vtc = [vtc_pool.tile([128, 8, 16, 64], f16, tag=f"vtc{h}",
                             name=f"vtc{h}") for h in range(HPC)]
        dst = dst_pool.tile([128, SL], f16, tag="dst")  # [hd, s*128+l]

        qv = q2[:].rearrange("p (s l) -> p s l", l=L)
        kv = k2[:].rearrange("p (s l) -> p s l", l=L)

        # ---- stage A (vt_row / vt_col built as v-slabs complete) ----
        with tc.tile_pool(name="vring", bufs=2) as vring, \
             tc.tile_pool(name="minip", bufs=1) as mini_pool, \
             tc.tile_pool(name="xload", bufs=2) as xpool, \
             tc.tile_pool(name="qkvps", bufs=4, space="PSUM") as qkv_ps:
            w_sb = xpool.tile([128, 4, 384], f16, tag="w", bufs=1)
            nc.sync.dma_start(w_sb[:],
                              w_d.ap().rearrange("(k c) o -> c k o", k=4))
            b_sb = xpool.tile([128, 3], f32, tag="b", bufs=1)
            nc.sync.dma_start(b_sb[:],
                              b_d.ap().rearrange("(m p) -> p m", p=128))
            slab = None
            for g in range(NG):
                if g % 4 == 0:
                    slab = vring.tile([128, 4 * GW], f16, tag="vslab",
                                      name="vslab")
                xt = xpool.tile([128, 4, GW], f16, tag="x")
                nc.gpsimd.dma_start(
                    xt[:],
                    x_flat[:, g * GW:(g + 1) * GW]
                        .rearrange("(k c) n -> c k n", k=4))
                for m in range(3):  # 0=q, 1=k, 2=v
                    dest = (q2, k2, slab)[m]
                    kind = "qkv"[m]
                    for sg in range(GW // 512):
                        ps = qkv_ps.tile([128, 512], f32, tag="acc",
                                         name="ps")
                        for c in range(4):
                            nc.tensor.matmul(
                                ps[:],
                                w_sb[:][:, c, m * 128:(m + 1) * 128],
                                xt[:][:, c, sg * 512:(sg + 1) * 512],
                                start=(c == 0), stop=(c == 3))
                        off = g * GW + sg * 512 if m < 2 else sg * 512
                        evac(kind, g * 4 + sg,
                             dest[:][:, off:off + 512], ps[:],
                             b_sb[:][:, m:m + 1])
                if g % 4 == 3:
                    # slab qtr complete: s in [32*qtr, 32*qtr+32)
                    qtr = g // 4
                    for h in range(HPC):
                        nc.sync.dma_start(
                            vtr[h][:][:, qtr],
                            slab[:][h * 64:(h + 1) * 64, :],
                            transpose=True)
                    for ql in range(4):
                        # mini[p, lr, sr] = slab[p, sr*128 + ql*32 + lr]
                        mini = mini_pool.tile([128, 32, 32], f16, tag="mini",
                                              name="mini")
                        msrc = AP(slab[:].tensor, slab[:].offset + ql * 32,
                                  [list(slab[:].ap[0]), [1, 32], [128, 32]])
                        nc.gpsimd.tensor_copy(mini[:], msrc)
                        for h in range(HPC):
                            # -> vtc[h][32-stripe @ qtr*32, ql, lr, d]
                            nc.sync.dma_start(
                                vtc[h][:][qtr * 32:(qtr + 1) * 32, ql, :, :],
                                mini[:][h * 64:(h + 1) * 64, :, :],
                                transpose=True)
        with tc.tile_pool(name="scps", bufs=2, space="PSUM") as sc_ps, \
             tc.tile_pool(name="avps", bufs=2, space="PSUM") as av_ps, \
             tc.tile_pool(name="trp", bufs=2) as tr_pool:
            pools = (src_pool, tr_pool, sc_ps, av_ps, e_pool, den_pool)
            if stage in ("row", "full"):
                for ch in range(NCH):
                    attention_chunk(1, ch, vtr, qv, kv, dst, zero_sb,
                                    ones_sb, pools)
            if stage == "full":
                for ch in range(NCH):
                    attention_chunk(0, ch, vtc, qv, kv, dst, zero_sb,
                                    ones_sb, pools)
            if stage == "a":
                nc.vector.tensor_copy(dst[:], q2[:])
            for sb in range(4):
                nc.sync.dma_start(
                    out_flat[:, sb * 32 * 128:(sb + 1) * 32 * 128],
                    dst[:][:, sb * 32 * 128:(sb + 1) * 32 * 128])


def _get_nc():
    if "nc" in _CACHE:
        return _CACHE["nc"]
    import concourse.bacc as bacc
    import concourse.tile as tile

    nc = bacc.Bacc(None, target_bir_lowering=False, debug=False,
                   num_devices=N_CORES)
    with tile.TileContext(nc) as tc:
        build_program(nc, tc)
    nc.compile()
    _CACHE["nc"] = nc
    return nc


def make_in_maps(x, W, b):
    x = np.asarray(x, dtype=np.float32)
    W = np.asarray(W, dtype=np.float32)
    b = np.asarray(b, dtype=np.float32)
    scale = np.float32(DIM_HEAD ** -0.5)
    in_maps = []
    for c in range(N_CORES):
        bb, h0 = c // 4, 2 * (c % 4)
        hd = np.arange(h0 * 64, (h0 + 2) * 64)
        sel = np.concatenate([hd, EMBED + hd, 2 * EMBED + hd])
        W_loc = W[sel, :].copy()
        b_loc = b[sel].copy()
        W_loc[:128] *= scale
        b_loc[:128] *= scale
        in_maps.append({
            "x": np.ascontiguousarray(x[bb]),
            "wT": np.ascontiguousarray(W_loc.T).astype(np.float16),
            "bvec": b_loc.astype(np.float32),
        })
    return in_maps


def assemble(results):
    out = np.empty((B, EMBED, S, L), dtype=np.float32)
    for c, r in enumerate(results):
        bb, h0 = c // 4, 2 * (c % 4)
        out[bb, h0 * 64:(h0 + 2) * 64] = r["out"].astype(np.float32)
    return out


def kernel(x, W, b):
    from concourse.bass_utils import run_bass_kernel_spmd
    nc = _get_nc()
    res = run_bass_kernel_spmd(nc, make_in_maps(x, W, b),
                               core_ids=list(range(N_CORES)))
    return assemble(res.results)
